# revision 1
# baseline (speedup 1.0000x reference)
"""HSTU block kernel for 8 Trainium2 NeuronCores.

Sharding: token-parallel. Core c handles batch b=c//4, tokens
[(c%4)*512, (c%4+1)*512). f1/attention/LN/f2 all computed locally for the
core's 512 query tokens; k/v for the full 2048-token batch are exchanged
with one AllGather per 4-core group.

Dataflow is feature-major (features on partitions) so the only transposes
are the initial x -> xT (32 PE transposes). LayerNorm over the feature dim
uses a ones-column matmul for the partition reduction and a K=1 ones-row
matmul to broadcast per-token stats back across partitions. The reference's
silu(scores)/S scaling is folded into LayerNorm via eps' = S^2 * eps
(LN is scale-invariant except for eps).

All big matmuls run in float32r (~13-bit mantissa, full PE rate).
"""

import sys

sys.path.insert(0, "/opt/trn_rl_repo")

import ml_dtypes
import numpy as np

import concourse.bass as bass
import concourse.mybir as mybir
import concourse.tile as tile
from concourse import bacc
from concourse.bass_utils import run_bass_kernel_spmd
from concourse.masks import make_identity

F32 = mybir.dt.float32
F32R = mybir.dt.float32r
BF16 = mybir.dt.bfloat16
SILU = mybir.ActivationFunctionType.Silu
SQRT = mybir.ActivationFunctionType.Sqrt
MULT = mybir.AluOpType.mult
ADD = mybir.AluOpType.add
SUB = mybir.AluOpType.subtract

B, S, D = 2, 2048, 1024
H, DH = 16, 64
T = 512            # tokens per core
NT = T // 128      # 4 token tiles per core
KC = D // 128      # 8 contraction chunks
NP = 8             # head pairs
EPS_EFF = float(S) * float(S) * 1e-5

_CACHE = {}


def _build():
    nc = bacc.Bacc(None, target_bir_lowering=False, num_devices=8)

    x_s = nc.dram_tensor("x_s", [T, D], F32, kind="ExternalInput")
    W1 = nc.dram_tensor("W1", [D, 4 * D], F32R, kind="ExternalInput")
    b1 = nc.dram_tensor("b1", [4 * D], F32, kind="ExternalInput")
    W2 = nc.dram_tensor("W2", [D, D], F32R, kind="ExternalInput")
    b2 = nc.dram_tensor("b2", [D], F32R, kind="ExternalInput")
    gamma = nc.dram_tensor("gamma", [D], F32, kind="ExternalInput")
    beta = nc.dram_tensor("beta", [D], F32, kind="ExternalInput")
    y_s = nc.dram_tensor("y_s", [T, D], F32, kind="ExternalOutput")

    # W1 column blocks: u [0:D], v [D:2D], q [2D:3D], k [3D:4D]
    U0, V0, Q0, K0 = 0, D, 2 * D, 3 * D

    with tile.TileContext(nc) as tc:
        with (
            tc.tile_pool(name="persist", bufs=1) as sbp,
            tc.tile_pool(name="small", bufs=2) as sbs,
            tc.tile_pool(name="dram", bufs=1, space="DRAM") as dram,
        ):
            # ---- constants
            ident = sbp.tile([128, 128], F32)
            make_identity(nc, ident[:])
            ones_f = sbp.tile([128, 128], F32)
            nc.vector.memset(ones_f[:], 1.0)
            ones_col = sbp.tile([128, 1], F32R)
            nc.vector.tensor_copy(ones_col[:], ones_f[:, 0:1])
            ones_row = sbp.tile([1, 128], F32R)
            nc.vector.tensor_copy(ones_row[:], ones_f[0:1, :])

            b1q = sbp.tile([128, 8], F32)
            b1k = sbp.tile([128, 8], F32)
            b1u = sbp.tile([128, 8], F32)
            nc.sync.dma_start(b1q[:], b1[Q0:Q0 + D].rearrange("(c p) -> p c", p=128))
            nc.sync.dma_start(b1k[:], b1[K0:K0 + D].rearrange("(c p) -> p c", p=128))
            nc.sync.dma_start(b1u[:], b1[U0:U0 + D].rearrange("(c p) -> p c", p=128))
            gam = sbp.tile([128, 8], F32)
            bet = sbp.tile([128, 8], F32)
            nc.sync.dma_start(gam[:], gamma[:].rearrange("(c p) -> p c", p=128))
            nc.sync.dma_start(bet[:], beta[:].rearrange("(c p) -> p c", p=128))

            b1v_row = sbp.tile([1, D], F32R)
            nc.sync.dma_start(b1v_row[:], b1[V0:V0 + D][None, :].bitcast(F32R))
            b2_row = sbp.tile([1, D], F32R)
            nc.sync.dma_start(b2_row[:], b2[:][None, :])

            # broadcast b1v / b2 across partitions via K=1 ones matmul
            b1v_sb = sbp.tile([128, D], F32)
            b2_sb = sbp.tile([128, D], F32)
            with tc.tile_pool(name="ps_bc", bufs=2, space="PSUM") as ps_bc:
                for nf in range(2):
                    pb = ps_bc.tile([128, 512], F32, tag="bc")
                    nc.tensor.matmul(pb[:], ones_row[:], b1v_row[:, nf * 512:(nf + 1) * 512],
                                     start=True, stop=True)
                    nc.vector.tensor_copy(b1v_sb[:, nf * 512:(nf + 1) * 512], pb[:])
                for nf in range(2):
                    pb = ps_bc.tile([128, 512], F32, tag="bc")
                    nc.tensor.matmul(pb[:], ones_row[:], b2_row[:, nf * 512:(nf + 1) * 512],
                                     start=True, stop=True)
                    nc.vector.tensor_copy(b2_sb[:, nf * 512:(nf + 1) * 512], pb[:])

            # ---- persistent activations
            xT = sbp.tile([128, KC, T], F32R)        # x^T, d on partitions
            qT = sbp.tile([128, NP, T], BF16)
            uT = sbp.tile([128, NP, T], F32)
            gatedT = sbp.tile([128, KC, T], F32R)
            normedT = sbp.tile([128, KC, T], F32R)

            # AG bounce buffers
            kv_in = dram.tile([128, 16, T], BF16)
            kv_out = dram.tile([512, 16, T], BF16)

            # ================= stage 0: load + transpose x =================
            with (
                tc.tile_pool(name="xload", bufs=2) as xload,
                tc.tile_pool(name="ps_tr", bufs=4, space="PSUM") as ps_tr,
            ):
                for tt in range(NT):
                    xa = xload.tile([128, D], F32, tag="xa")
                    nc.sync.dma_start(xa[:], x_s[tt * 128:(tt + 1) * 128, :])
                    for kc in range(KC):
                        pt = ps_tr.tile([128, 128], F32, tag="tr")
                        nc.tensor.transpose(pt[:], xa[:, kc * 128:(kc + 1) * 128], ident[:])
                        nc.vector.tensor_copy(xT[:, kc, tt * 128:(tt + 1) * 128], pt[:])

            # ================= stage 1: f1 =================
            with (
                tc.tile_pool(name="w1pool", bufs=12) as w1pool,
                tc.tile_pool(name="wvpool", bufs=2) as wvpool,
                tc.tile_pool(name="kvloc", bufs=1) as kvloc,
            ):
                kT_loc = kvloc.tile([128, NP, T], BF16)
                v_loc = kvloc.tile([128, NT, D], BF16)

                # k (feature-major) -> kT_loc
                with tc.tile_pool(name="ps_k", bufs=2, space="PSUM") as ps_k:
                  for hc in range(NP):
                    ps = ps_k.tile([128, T], F32, tag="f1")
                    for kc in range(KC):
                        wb = w1pool.tile([128, 128], F32R, tag="w1blk")
                        nc.sync.dma_start(
                            wb[:], W1[kc * 128:(kc + 1) * 128, K0 + hc * 128:K0 + (hc + 1) * 128])
                        nc.tensor.matmul(ps[:], wb[:], xT[:, kc, :],
                                         start=(kc == 0), stop=(kc == KC - 1))
                    nc.scalar.activation(kT_loc[:, hc, :], ps[:], SILU,
                                         bias=b1k[:, hc:hc + 1], scale=1.0)
                nc.gpsimd.dma_start(kv_in[:, 0:8, :], kT_loc[:])

                # v (token-major) -> v_loc; kc outer so each xT lhsT load
                # feeds both nf matmuls
                with tc.tile_pool(name="ps_v", bufs=1, space="PSUM") as ps_v:
                  psv = [ps_v.tile([128, 1024], F32, tag=f"v{tt}", name=f"psv{tt}")
                         for tt in range(NT)]
                  for kc in range(KC):
                    wv = wvpool.tile([128, 1024], F32R, tag="wv")
                    nc.sync.dma_start(wv[:], W1[kc * 128:(kc + 1) * 128, V0:V0 + D])
                    for tt in range(NT):
                        for nf in range(2):
                            nc.tensor.matmul(psv[tt][:, nf * 512:(nf + 1) * 512],
                                             xT[:, kc, tt * 128:(tt + 1) * 128],
                                             wv[:, nf * 512:(nf + 1) * 512],
                                             start=(kc == 0), stop=(kc == KC - 1))
                  for tt in range(NT):
                    vt = sbs.tile([128, 1024], F32, tag="vtmp")
                    nc.vector.tensor_tensor(vt[:], psv[tt][:], b1v_sb[:], ADD)
                    nc.scalar.activation(v_loc[:, tt, :], vt[:], SILU)
                nc.gpsimd.dma_start(
                    kv_in[:, 8:16, :],
                    v_loc[:].rearrange("p tt (h f) -> p (tt h) f", h=2))
                tc.no_sync_barrier()

                # q, u (overlap the AllGather)
                with tc.tile_pool(name="ps_qu", bufs=2, space="PSUM") as ps_qu:
                  for hc in range(NP):
                    ps = ps_qu.tile([128, T], F32, tag="f1")
                    for kc in range(KC):
                        wb = w1pool.tile([128, 128], F32R, tag="w1blk")
                        nc.sync.dma_start(
                            wb[:], W1[kc * 128:(kc + 1) * 128, Q0 + hc * 128:Q0 + (hc + 1) * 128])
                        nc.tensor.matmul(ps[:], wb[:], xT[:, kc, :],
                                         start=(kc == 0), stop=(kc == KC - 1))
                    nc.scalar.activation(qT[:, hc, :], ps[:], SILU,
                                         bias=b1q[:, hc:hc + 1], scale=1.0)
                  for hc in range(NP):
                    ps = ps_qu.tile([128, T], F32, tag="f1")
                    for kc in range(KC):
                        wb = w1pool.tile([128, 128], F32R, tag="w1blk")
                        nc.sync.dma_start(
                            wb[:], W1[kc * 128:(kc + 1) * 128, U0 + hc * 128:U0 + (hc + 1) * 128])
                        nc.tensor.matmul(ps[:], wb[:], xT[:, kc, :],
                                         start=(kc == 0), stop=(kc == KC - 1))
                    nc.scalar.activation(uT[:, hc, :], ps[:], SILU,
                                         bias=b1u[:, hc:hc + 1], scale=1.0)

                # single AllGather for k+v within each 4-core group
                nc.gpsimd.collective_compute(
                    "AllGather", mybir.AluOpType.bypass,
                    replica_groups=[[0, 1, 2, 3], [4, 5, 6, 7]],
                    ins=[kv_in[:]], outs=[kv_out[:]])

            # ================= stage 2: attention per head pair =================
            with (
                tc.tile_pool(name="kvfull", bufs=2) as kvfull,
                tc.tile_pool(name="attn", bufs=2) as attn,
                tc.tile_pool(name="ps_s", bufs=1, space="PSUM") as ps_s,
                tc.tile_pool(name="ps_av", bufs=2, space="PSUM") as ps_av,
            ):
                for hc in range(NP):
                    ktf = kvfull.tile([128, 2048], BF16, tag="ktf")
                    for r in range(4):
                        nc.sync.dma_start(ktf[:, r * 512:(r + 1) * 512],
                                          kv_out[r * 128:(r + 1) * 128, hc, :])
                    vf = kvfull.tile([128, 16, 128], BF16, tag="vf")
                    for r in range(4):
                        for tt in range(NT):
                            nc.sync.dma_start(
                                vf[:, r * 4 + tt, :],
                                kv_out[r * 128:(r + 1) * 128, 8 + tt * 2 + hc // 4,
                                       (hc % 4) * 128:(hc % 4) * 128 + 128])

                    av0 = ps_av.tile([128, 512], F32, tag="av0")
                    av1 = ps_av.tile([128, 512], F32, tag="av1")
                    for kg in range(8):
                        s0 = ps_s.tile([128, 1024], F32, tag="s0")
                        s1 = ps_s.tile([128, 1024], F32, tag="s1")
                        for sub in range(2):
                            ktc = kg * 2 + sub
                            nc.tensor.matmul(s0[:, sub * 512:(sub + 1) * 512],
                                             ktf[0:64, ktc * 128:(ktc + 1) * 128],
                                             qT[0:64, hc, :], start=True, stop=True)
                            nc.tensor.matmul(s1[:, sub * 512:(sub + 1) * 512],
                                             ktf[64:128, ktc * 128:(ktc + 1) * 128],
                                             qT[64:128, hc, :], start=True, stop=True,
                                             tile_position=(64, 0))
                        a0 = attn.tile([128, 1024], BF16, tag="a0")
                        a1 = attn.tile([128, 1024], BF16, tag="a1")
                        nc.scalar.activation(a0[:], s0[:], SILU)
                        nc.scalar.activation(a1[:], s1[:], SILU)
                        for sub in range(2):
                            ktc = kg * 2 + sub
                            # full-width lhsT: head0 valid rows 0:64, head1 rows 64:128
                            nc.tensor.matmul(av0[:], vf[:, ktc, :],
                                             a0[:, sub * 512:(sub + 1) * 512],
                                             start=(ktc == 0), stop=(ktc == 15))
                            nc.tensor.matmul(av1[:], vf[:, ktc, :],
                                             a1[:, sub * 512:(sub + 1) * 512],
                                             start=(ktc == 0), stop=(ktc == 15))
                    nc.vector.tensor_tensor(gatedT[0:64, hc, :], av0[0:64, :],
                                            uT[0:64, hc, :], MULT)
                    nc.vector.tensor_tensor(gatedT[64:128, hc, :], av1[64:128, :],
                                            uT[64:128, hc, :], MULT)

            # ================= stage 3: LayerNorm =================
            with (
                tc.tile_pool(name="ln", bufs=2) as ln,
                tc.tile_pool(name="ps_ln", bufs=1, space="PSUM") as ps_ln,
            ):
                st_sum = ps_ln.tile([1, T], F32, tag="st_sum")
                st_sq = ps_ln.tile([1, T], F32, tag="st_sq")
                for kc in range(KC):
                    nc.tensor.matmul(st_sum[:], ones_col[:], gatedT[:, kc, :],
                                     start=(kc == 0), stop=(kc == KC - 1))
                for kc in range(KC):
                    sq = ln.tile([128, T], F32R, tag="sq")
                    nc.vector.tensor_tensor(sq[:], gatedT[:, kc, :].bitcast(F32),
                                            gatedT[:, kc, :].bitcast(F32), MULT)
                    nc.tensor.matmul(st_sq[:], ones_col[:], sq[:],
                                     start=(kc == 0), stop=(kc == KC - 1))

                mu = ln.tile([1, T], F32, tag="mu")
                nc.vector.tensor_scalar_mul(mu[:], st_sum[:], 1.0 / D)
                m2 = ln.tile([1, T], F32, tag="m2")
                nc.vector.tensor_scalar_mul(m2[:], st_sq[:], 1.0 / D)
                mu2 = ln.tile([1, T], F32, tag="mu2")
                nc.vector.tensor_tensor(mu2[:], mu[:], mu[:], MULT)
                varE = ln.tile([1, T], F32, tag="varE")
                nc.vector.tensor_tensor(varE[:], m2[:], mu2[:], SUB)
                nc.vector.tensor_scalar_add(varE[:], varE[:], EPS_EFF)
                std = ln.tile([1, T], F32, tag="std")
                nc.scalar.activation(std[:], varE[:], SQRT)
                r0 = ln.tile([1, T], F32, tag="r0")
                nc.vector.reciprocal(r0[:], std[:])
                # one Newton step: r1 = r0 * (1.5 - 0.5 * varE * r0^2)
                nt1 = ln.tile([1, T], F32, tag="nt1")
                nc.vector.tensor_tensor(nt1[:], r0[:], r0[:], MULT)
                nc.vector.tensor_tensor(nt1[:], nt1[:], varE[:], MULT)
                nc.vector.tensor_scalar(nt1[:], nt1[:], -0.5, 1.5, MULT, ADD)
                rstd = ln.tile([1, T], F32R, tag="rstd")
                nc.vector.tensor_tensor(rstd[:], r0[:], nt1[:], MULT)
                mu_r = ln.tile([1, T], F32R, tag="mu_r")
                nc.vector.tensor_copy(mu_r[:], mu[:])

                ps_mu = ps_ln.tile([128, T], F32, tag="ps_mu")
                ps_r = ps_ln.tile([128, T], F32, tag="ps_r")
                nc.tensor.matmul(ps_mu[:], ones_row[:], mu_r[:], start=True, stop=True)
                nc.tensor.matmul(ps_r[:], ones_row[:], rstd[:], start=True, stop=True)

                for kc in range(KC):
                    t1 = ln.tile([128, T], F32, tag="t1")
                    nc.vector.tensor_tensor(t1[:], gatedT[:, kc, :].bitcast(F32), ps_mu[:], SUB)
                    nc.vector.tensor_tensor(t1[:], t1[:], ps_r[:], MULT)
                    nc.vector.tensor_scalar(normedT[:, kc, :], t1[:],
                                            gam[:, kc:kc + 1], bet[:, kc:kc + 1], MULT, ADD)

            # ================= stage 4: f2 + bias + store =================
            with (
                tc.tile_pool(name="w2pool", bufs=4) as w2pool,
                tc.tile_pool(name="yout", bufs=2) as yout,
                tc.tile_pool(name="ps_y", bufs=1, space="PSUM") as ps_y,
            ):
                for nf in range(2):
                    psy = [ps_y.tile([128, 512], F32, tag=f"y{tt}", name=f"psy{tt}") for tt in range(NT)]
                    for kc in range(KC):
                        w2b = w2pool.tile([128, 512], F32R, tag="w2b")
                        nc.sync.dma_start(
                            w2b[:], W2[kc * 128:(kc + 1) * 128, nf * 512:(nf + 1) * 512])
                        for tt in range(NT):
                            nc.tensor.matmul(psy[tt][:], normedT[:, kc, tt * 128:(tt + 1) * 128],
                                             w2b[:], start=(kc == 0), stop=(kc == KC - 1))
                    for tt in range(NT):
                        yo = yout.tile([128, 512], F32, tag="yo")
                        nc.vector.tensor_tensor(yo[:], psy[tt][:],
                                                b2_sb[:, nf * 512:(nf + 1) * 512], ADD)
                        nc.sync.dma_start(
                            y_s[tt * 128:(tt + 1) * 128, nf * 512:(nf + 1) * 512], yo[:])

    nc.compile()
    return nc


def _get_nc():
    if "nc" not in _CACHE:
        _CACHE["nc"] = _build()
    return _CACHE["nc"]


def kernel(x, W1, b1, W2, b2, gamma, beta, **kw):
    nc = _get_nc()
    x = np.ascontiguousarray(x, dtype=np.float32)
    in_maps = []
    for c in range(8):
        b = c // 4
        t0 = (c % 4) * T
        in_maps.append({
            "x_s": np.ascontiguousarray(x[b, t0:t0 + T, :]),
            "W1": np.ascontiguousarray(W1, dtype=np.float32),
            "b1": np.ascontiguousarray(b1, dtype=np.float32),
            "W2": np.ascontiguousarray(W2, dtype=np.float32),
            "b2": np.ascontiguousarray(b2, dtype=np.float32),
            "gamma": np.ascontiguousarray(gamma, dtype=np.float32),
            "beta": np.ascontiguousarray(beta, dtype=np.float32),
        })
    res = run_bass_kernel_spmd(nc, in_maps, core_ids=list(range(8)), **kw)
    y = np.empty((B, S, D), dtype=np.float32)
    for c in range(8):
        b = c // 4
        t0 = (c % 4) * T
        y[b, t0:t0 + T, :] = res.results[c]["y_s"]
    if kw:
        _CACHE["last_res"] = res
    return y



# revision 6
# speedup vs baseline: 1.2682x; 1.2682x over previous
"""HSTU block kernel for 8 Trainium2 NeuronCores — head-sharded attention.

Sharding: core c owns heads {2c, 2c+1} (a 128-feature slice of each of the
q/k/v/u projections) for ALL 4096 tokens (both batches). f1 computes only
those W1 columns, so attention is fully local per core — no kv exchange.
After gating (av*u, feature-major) one 8-core AllToAll (1 MB) re-shards
features -> tokens; LN + f2 then run token-parallel on 512 tokens per core.

All matmuls run in bf16 (full PE rate, half the DMA/SBUF of fp32). The host
pre-transposes x and pre-slices W1/b1 so the device does no input transposes
and only ~20 fat DMAs. Scores use K=64 contraction with the two heads of a
pair packed on partitions via tile_position; AV accumulates both heads into
one PSUM tile via output tile_position. The silu(scores)/S scaling is folded
into LayerNorm via eps' = S^2 * eps (LN is scale-invariant except for eps).
"""

import sys

sys.path.insert(0, "/opt/trn_rl_repo")

import ml_dtypes
import numpy as np

import concourse.bass as bass
import concourse.mybir as mybir
import concourse.tile as tile
from concourse import bacc
from concourse.bass_utils import run_bass_kernel_spmd
from concourse.masks import make_identity

F32 = mybir.dt.float32
F32R = mybir.dt.float32r
BF16 = mybir.dt.bfloat16
SILU = mybir.ActivationFunctionType.Silu
SQRT = mybir.ActivationFunctionType.Sqrt
MULT = mybir.AluOpType.mult
ADD = mybir.AluOpType.add
SUB = mybir.AluOpType.subtract

B, S, D = 2, 2048, 1024
TT = B * S         # 4096 tokens total (f1/attention scope per core)
TO = 512           # output tokens per core
KC = D // 128      # 8 feature chunks of the model dim
NTC = TT // 128    # 32 token chunks
EPS_EFF = float(S) * float(S) * 1e-5

_CACHE = {}


def _build():
    nc = bacc.Bacc(None, target_bir_lowering=False, num_devices=8)

    xT = nc.dram_tensor("xT", [D, TT], BF16, kind="ExternalInput")
    w1q = nc.dram_tensor("w1q", [D, 128], BF16, kind="ExternalInput")
    w1k = nc.dram_tensor("w1k", [D, 128], BF16, kind="ExternalInput")
    w1v = nc.dram_tensor("w1v", [D, 128], BF16, kind="ExternalInput")
    w1u = nc.dram_tensor("w1u", [D, 128], BF16, kind="ExternalInput")
    b1q = nc.dram_tensor("b1q", [128, 1], F32, kind="ExternalInput")
    b1k = nc.dram_tensor("b1k", [128, 1], F32, kind="ExternalInput")
    b1v = nc.dram_tensor("b1v", [128, 1], F32, kind="ExternalInput")
    b1u = nc.dram_tensor("b1u", [128, 1], F32, kind="ExternalInput")
    w2 = nc.dram_tensor("w2", [D, D], BF16, kind="ExternalInput")
    b2_bc = nc.dram_tensor("b2_bc", [128, D], F32, kind="ExternalInput")
    gam_in = nc.dram_tensor("gam", [128, KC], F32, kind="ExternalInput")
    bet_in = nc.dram_tensor("bet", [128, KC], F32, kind="ExternalInput")
    y_s = nc.dram_tensor("y_s", [TO, D], F32, kind="ExternalOutput")

    xT_v = xT.rearrange("(kc p) t -> p kc t", p=128)
    w1q_v = w1q.rearrange("(kc p) f -> p kc f", p=128)
    w1k_v = w1k.rearrange("(kc p) f -> p kc f", p=128)
    w1v_v = w1v.rearrange("(kc p) f -> p kc f", p=128)
    w1u_v = w1u.rearrange("(kc p) f -> p kc f", p=128)
    w2_v = w2.rearrange("(kc p) n -> p kc n", p=128)

    with tile.TileContext(nc) as tc:
        with (
            tc.tile_pool(name="persist", bufs=1) as sbp,
            tc.tile_pool(name="small", bufs=2) as sbs,
            tc.tile_pool(name="dram", bufs=1, space="DRAM") as dram,
        ):
            # ---- constants / small params
            ident = sbp.tile([128, 128], BF16)
            make_identity(nc, ident[:])
            ones_f = sbp.tile([128, 128], F32)
            nc.vector.memset(ones_f[:], 1.0)
            ones_col = sbp.tile([128, 1], BF16)
            nc.vector.tensor_copy(ones_col[:], ones_f[:, 0:1])
            ones_row = sbp.tile([1, 128], F32R)
            nc.vector.tensor_copy(ones_row[:], ones_f[0:1, :])

            b1q_s = sbp.tile([128, 1], F32)
            b1k_s = sbp.tile([128, 1], F32)
            b1v_s = sbp.tile([128, 1], F32)
            b1u_s = sbp.tile([128, 1], F32)
            b2_s = sbp.tile([128, D], F32)
            gam = sbp.tile([128, KC], F32)
            bet = sbp.tile([128, KC], F32)
            nc.sync.dma_start(b1q_s[:], b1q[:, :])
            nc.sync.dma_start(b1k_s[:], b1k[:, :])
            nc.sync.dma_start(b1v_s[:], b1v[:, :])
            nc.sync.dma_start(b1u_s[:], b1u[:, :])
            nc.sync.dma_start(b2_s[:], b2_bc[:, :])
            nc.sync.dma_start(gam[:], gam_in[:, :])
            nc.sync.dma_start(bet[:], bet_in[:, :])

            # ---- persistent activations (bf16)
            xTs = sbp.tile([128, KC, TT], BF16)      # 64 KB/partition
            kT = sbp.tile([128, TT], BF16)
            qT = sbp.tile([128, TT], BF16)
            uT = sbp.tile([128, TT], BF16)
            vTok = sbp.tile([128, NTC, 128], BF16)   # v token-major
            gatedT = sbp.tile([128, TT], BF16)
            gfull = sbp.tile([128, KC, TO], BF16)
            normedT = sbp.tile([128, KC, TO], BF16)

            w1q_t = sbp.tile([128, KC, 128], BF16)
            w1k_t = sbp.tile([128, KC, 128], BF16)
            w1v_t = sbp.tile([128, KC, 128], BF16)
            w1u_t = sbp.tile([128, KC, 128], BF16)
            w2_t = sbp.tile([128, KC, D], BF16)

            a2a_in = dram.tile([8, 128, TO], BF16, name="a2a_in")
            a2a_out = dram.tile([8, 128, TO], BF16, name="a2a_out")

            # x in four token-quarters so f1 can start early
            for qh in range(4):
                nc.sync.dma_start(xTs[:, :, qh * 1024:(qh + 1) * 1024],
                                  xT_v[:, :, qh * 1024:(qh + 1) * 1024])
            nc.sync.dma_start(w1k_t[:], w1k_v[:, :, :])
            nc.sync.dma_start(w1q_t[:], w1q_v[:, :, :])
            nc.sync.dma_start(w1v_t[:], w1v_v[:, :, :])
            nc.sync.dma_start(w1u_t[:], w1u_v[:, :, :])

            # ================= stage 1: f1 (k, q, v, u) =================
            with (
                tc.tile_pool(name="ps_f1", bufs=2, space="PSUM") as ps_f1,
                tc.tile_pool(name="ps_tr", bufs=4, space="PSUM") as ps_tr,
            ):
                # feature-major projections: out [128 f, 1024 t] per quarter
                for wt, bt, dst in ((w1k_t, b1k_s, kT), (w1q_t, b1q_s, qT),
                                    (w1v_t, b1v_s, None), (w1u_t, b1u_s, uT)):
                    for qh in range(4):
                        ps = ps_f1.tile([128, 1024], F32, tag="f1")
                        for kc in range(KC):
                            for n2 in range(2):
                                nc.tensor.matmul(
                                    ps[:, n2 * 512:(n2 + 1) * 512], wt[:, kc, :],
                                    xTs[:, kc, qh * 1024 + n2 * 512:qh * 1024 + (n2 + 1) * 512],
                                    start=(kc == 0), stop=(kc == KC - 1))
                        if dst is not None:
                            nc.scalar.activation(
                                dst[:, qh * 1024:(qh + 1) * 1024], ps[:],
                                SILU, bias=bt[:, 0:1], scale=1.0)
                        else:
                            # v: silu into a staging tile, then PE-transpose
                            vf = sbs.tile([128, 1024], BF16, tag="vf")
                            nc.scalar.activation(vf[:], ps[:], SILU,
                                                 bias=bt[:, 0:1], scale=1.0)
                            for t8 in range(8):
                                pt = ps_tr.tile([128, 128], BF16, tag="tr")
                                nc.tensor.transpose(
                                    pt[:], vf[:, t8 * 128:(t8 + 1) * 128],
                                    ident[:])
                                nc.vector.tensor_copy(
                                    vTok[:, qh * 8 + t8, :], pt[:])

            # ================= stage 2: attention per batch =================
            with (
                tc.tile_pool(name="ps_av", bufs=1, space="PSUM") as ps_av,
                tc.tile_pool(name="ps_s", bufs=2, space="PSUM") as ps_s,
                tc.tile_pool(name="attn", bufs=3) as attn,
            ):
                for bt in range(2):
                    t_b = bt * S
                    av = ps_av.tile([128, 2, 1024], F32, tag="av")
                    for qh in range(2):
                        q0 = t_b + qh * 1024
                        for kt in range(16):
                            t0 = t_b + kt * 128
                            sa = ps_s.tile([128, 1024], F32, tag="s")
                            for n2 in range(2):
                                nc.tensor.matmul(
                                    sa[:, n2 * 512:(n2 + 1) * 512],
                                    kT[0:64, t0:t0 + 128],
                                    qT[0:64, q0 + n2 * 512:q0 + (n2 + 1) * 512],
                                    start=True, stop=True)
                            aa = attn.tile([128, 1024], BF16, tag="a")
                            nc.scalar.activation(aa[:], sa[:], SILU)
                            for n2 in range(2):
                                nc.tensor.matmul(
                                    av[0:64, qh, n2 * 512:(n2 + 1) * 512],
                                    vTok[:, bt * 16 + kt, 0:64],
                                    aa[:, n2 * 512:(n2 + 1) * 512],
                                    start=(kt == 0), stop=(kt == 15))

                            sb = ps_s.tile([128, 1024], F32, tag="s")
                            for n2 in range(2):
                                nc.tensor.matmul(
                                    sb[:, n2 * 512:(n2 + 1) * 512],
                                    kT[64:128, t0:t0 + 128],
                                    qT[64:128, q0 + n2 * 512:q0 + (n2 + 1) * 512],
                                    start=True, stop=True,
                                    tile_position=(64, 0))
                            ab = attn.tile([128, 1024], BF16, tag="a")
                            nc.scalar.activation(ab[:], sb[:], SILU)
                            for n2 in range(2):
                                nc.tensor.matmul(
                                    av[64:128, qh, n2 * 512:(n2 + 1) * 512],
                                    vTok[:, bt * 16 + kt, 64:128],
                                    ab[:, n2 * 512:(n2 + 1) * 512],
                                    start=(kt == 0), stop=(kt == 15),
                                    tile_position=(0, 64))
                    # gate with u; ship the finished token range to DRAM
                    for qh in range(2):
                        g0 = t_b + qh * 1024
                        nc.vector.tensor_tensor(
                            gatedT[:, g0:g0 + 1024], av[:, qh, :],
                            uT[:, g0:g0 + 1024], MULT)
                        j0 = (bt * 2 + qh) * 2
                        nc.gpsimd.dma_start(a2a_in[j0],
                                            gatedT[:, g0:g0 + TO])
                        nc.gpsimd.dma_start(a2a_in[j0 + 1],
                                            gatedT[:, g0 + TO:g0 + 1024])

            # ================= stage 3: AllToAll + LayerNorm =================
            nc.gpsimd.collective_compute(
                "AllToAll", mybir.AluOpType.bypass,
                replica_groups=[[0, 1, 2, 3, 4, 5, 6, 7]],
                ins=[a2a_in[:]], outs=[a2a_out[:]])
            nc.sync.dma_start(gfull[:], a2a_out.rearrange("j p t -> p j t"))

            with (
                tc.tile_pool(name="ln", bufs=2) as ln,
                tc.tile_pool(name="ps_ln", bufs=1, space="PSUM") as ps_ln,
            ):
                st_sum = ps_ln.tile([1, TO], F32, tag="st_sum")
                st_sq = ps_ln.tile([1, TO], F32, tag="st_sq")
                for kc in range(KC):
                    nc.tensor.matmul(st_sum[:], ones_col[:], gfull[:, kc, :],
                                     start=(kc == 0), stop=(kc == KC - 1))
                for kc in range(KC):
                    sq = ln.tile([128, TO], BF16, tag="sq")
                    nc.vector.tensor_tensor(sq[:], gfull[:, kc, :],
                                            gfull[:, kc, :], MULT)
                    nc.tensor.matmul(st_sq[:], ones_col[:], sq[:],
                                     start=(kc == 0), stop=(kc == KC - 1))

                mu = ln.tile([1, TO], F32, tag="mu")
                nc.vector.tensor_scalar_mul(mu[:], st_sum[:], 1.0 / D)
                m2 = ln.tile([1, TO], F32, tag="m2")
                nc.vector.tensor_scalar_mul(m2[:], st_sq[:], 1.0 / D)
                mu2 = ln.tile([1, TO], F32, tag="mu2")
                nc.vector.tensor_tensor(mu2[:], mu[:], mu[:], MULT)
                varE = ln.tile([1, TO], F32, tag="varE")
                nc.vector.tensor_tensor(varE[:], m2[:], mu2[:], SUB)
                nc.vector.tensor_scalar_add(varE[:], varE[:], EPS_EFF)
                std = ln.tile([1, TO], F32, tag="std")
                nc.scalar.activation(std[:], varE[:], SQRT)
                r0 = ln.tile([1, TO], F32, tag="r0")
                nc.vector.reciprocal(r0[:], std[:])
                # one Newton step: r1 = r0 * (1.5 - 0.5 * varE * r0^2)
                nt1 = ln.tile([1, TO], F32, tag="nt1")
                nc.vector.tensor_tensor(nt1[:], r0[:], r0[:], MULT)
                nc.vector.tensor_tensor(nt1[:], nt1[:], varE[:], MULT)
                nc.vector.tensor_scalar(nt1[:], nt1[:], -0.5, 1.5, MULT, ADD)
                rstd = ln.tile([1, TO], F32R, tag="rstd")
                nc.vector.tensor_tensor(rstd[:], r0[:], nt1[:], MULT)
                mu_r = ln.tile([1, TO], F32R, tag="mu_r")
                nc.vector.tensor_copy(mu_r[:], mu[:])

                ps_mu = ps_ln.tile([128, TO], F32, tag="ps_mu")
                ps_r = ps_ln.tile([128, TO], F32, tag="ps_r")
                nc.tensor.matmul(ps_mu[:], ones_row[:], mu_r[:], start=True, stop=True)
                nc.tensor.matmul(ps_r[:], ones_row[:], rstd[:], start=True, stop=True)

                for kc in range(KC):
                    t1 = ln.tile([128, TO], F32, tag="t1")
                    nc.vector.tensor_tensor(t1[:], gfull[:, kc, :], ps_mu[:], SUB)
                    nc.vector.tensor_tensor(t1[:], t1[:], ps_r[:], MULT)
                    nc.vector.tensor_scalar(normedT[:, kc, :], t1[:],
                                            gam[:, kc:kc + 1], bet[:, kc:kc + 1],
                                            MULT, ADD)

                # load W2 while attention/LN runs
                nc.sync.dma_start(w2_t[:], w2_v[:, :, :])

            # ================= stage 4: f2 + bias + store =================
            with (
                tc.tile_pool(name="ps_y", bufs=2, space="PSUM") as ps_y,
                tc.tile_pool(name="yout", bufs=2) as yout,
            ):
                for tt in range(4):
                    psy = ps_y.tile([128, D], F32, tag="y")
                    for kc in range(KC):
                        for n2 in range(2):
                            nc.tensor.matmul(
                                psy[:, n2 * 512:(n2 + 1) * 512],
                                normedT[:, kc, tt * 128:(tt + 1) * 128],
                                w2_t[:, kc, n2 * 512:(n2 + 1) * 512],
                                start=(kc == 0), stop=(kc == KC - 1))
                    yo = yout.tile([128, D], F32, tag="yo")
                    nc.vector.tensor_tensor(yo[:], psy[:], b2_s[:], ADD)
                    nc.sync.dma_start(y_s[tt * 128:(tt + 1) * 128, :], yo[:])

    nc.compile()
    return nc


def _get_nc():
    if "nc" not in _CACHE:
        _CACHE["nc"] = _build()
    return _CACHE["nc"]


def kernel(x, W1, b1, W2, b2, gamma, beta, **kw):
    nc = _get_nc()
    bf16 = ml_dtypes.bfloat16
    x = np.asarray(x, dtype=np.float32)
    W1 = np.asarray(W1, dtype=np.float32)
    b1 = np.asarray(b1, dtype=np.float32)
    W2bf = np.ascontiguousarray(np.asarray(W2, dtype=np.float32).astype(bf16))
    b2_bc = np.ascontiguousarray(
        np.broadcast_to(np.asarray(b2, dtype=np.float32), (128, D)))
    gam_t = np.ascontiguousarray(
        np.asarray(gamma, dtype=np.float32).reshape(KC, 128).T)
    bet_t = np.ascontiguousarray(
        np.asarray(beta, dtype=np.float32).reshape(KC, 128).T)
    # [D, 4096]: both batches concatenated along tokens
    xTall = np.ascontiguousarray(
        x.transpose(2, 0, 1).reshape(D, TT).astype(bf16))

    in_maps = []
    u0, v0, q0, k0 = 0, D, 2 * D, 3 * D
    for c in range(8):
        cs = 128 * c
        in_maps.append({
            "xT": xTall,
            "w1q": np.ascontiguousarray(W1[:, q0 + cs:q0 + cs + 128].astype(bf16)),
            "w1k": np.ascontiguousarray(W1[:, k0 + cs:k0 + cs + 128].astype(bf16)),
            "w1v": np.ascontiguousarray(W1[:, v0 + cs:v0 + cs + 128].astype(bf16)),
            "w1u": np.ascontiguousarray(W1[:, u0 + cs:u0 + cs + 128].astype(bf16)),
            "b1q": np.ascontiguousarray(b1[q0 + cs:q0 + cs + 128].reshape(128, 1)),
            "b1k": np.ascontiguousarray(b1[k0 + cs:k0 + cs + 128].reshape(128, 1)),
            "b1v": np.ascontiguousarray(b1[v0 + cs:v0 + cs + 128].reshape(128, 1)),
            "b1u": np.ascontiguousarray(b1[u0 + cs:u0 + cs + 128].reshape(128, 1)),
            "w2": W2bf,
            "b2_bc": b2_bc,
            "gam": gam_t,
            "bet": bet_t,
        })
    res = run_bass_kernel_spmd(nc, in_maps, core_ids=list(range(8)), **kw)
    y = np.empty((B, S, D), dtype=np.float32)
    for c in range(8):
        b = c // 4
        t0 = 512 * (c % 4)
        y[b, t0:t0 + 512, :] = res.results[c]["y_s"]
    if kw:
        _CACHE["last_res"] = res
    return y


# revision 18
# speedup vs baseline: 1.5765x; 1.2431x over previous
"""HSTU block kernel for 8 Trainium2 NeuronCores — head-sharded attention.

Sharding: core c owns heads {2c, 2c+1} (a 128-feature slice of each of the
q/k/v/u projections) for ALL 4096 tokens (both batches). f1 computes only
those W1 columns, so attention is fully local per core — no kv exchange.
After gating (av*u, feature-major) one 8-core AllToAll (1 MB) re-shards
features -> tokens; LN + f2 then run token-parallel on 512 tokens per core.

All matmuls run in bf16 (full PE rate, half the DMA/SBUF of fp32). The host
pre-transposes x and pre-slices W1/b1 so the device does no input transposes
and only ~20 fat DMAs. Scores use K=64 contraction with the two heads of a
pair packed on partitions via tile_position; AV accumulates both heads into
one PSUM tile via output tile_position. The silu(scores)/S scaling is folded
into LayerNorm via eps' = S^2 * eps (LN is scale-invariant except for eps).
"""

import sys

sys.path.insert(0, "/opt/trn_rl_repo")

import ml_dtypes
import numpy as np

import concourse.bass as bass
import concourse.mybir as mybir
import concourse.tile as tile
from concourse import bacc
from concourse.bass_utils import run_bass_kernel_spmd
from concourse.masks import make_identity

F32 = mybir.dt.float32
F32R = mybir.dt.float32r
BF16 = mybir.dt.bfloat16
SILU = mybir.ActivationFunctionType.Silu
SQRT = mybir.ActivationFunctionType.Sqrt
MULT = mybir.AluOpType.mult
ADD = mybir.AluOpType.add
SUB = mybir.AluOpType.subtract

B, S, D = 2, 2048, 1024
TT = B * S         # 4096 tokens total (f1/attention scope per core)
TO = 512           # output tokens per core
KC = D // 128      # 8 feature chunks of the model dim
NTC = TT // 128    # 32 token chunks
EPS_EFF = float(S) * float(S) * 1e-5

_CACHE = {}


def _build():
    nc = bacc.Bacc(None, target_bir_lowering=False, num_devices=8)

    xT = nc.dram_tensor("xT", [D, TT], BF16, kind="ExternalInput")
    w1q = nc.dram_tensor("w1q", [D, 128], BF16, kind="ExternalInput")
    w1k = nc.dram_tensor("w1k", [D, 128], BF16, kind="ExternalInput")
    w1v = nc.dram_tensor("w1v", [D, 128], BF16, kind="ExternalInput")
    w1u = nc.dram_tensor("w1u", [D, 128], BF16, kind="ExternalInput")
    b1q = nc.dram_tensor("b1q", [128, 1], F32, kind="ExternalInput")
    b1k = nc.dram_tensor("b1k", [128, 1], F32, kind="ExternalInput")
    b1v = nc.dram_tensor("b1v", [128, 1], F32, kind="ExternalInput")
    b1u = nc.dram_tensor("b1u", [128, 1], F32, kind="ExternalInput")
    w2 = nc.dram_tensor("w2", [D, D], BF16, kind="ExternalInput")
    b2_bc = nc.dram_tensor("b2_bc", [128, D], F32, kind="ExternalInput")
    gam_in = nc.dram_tensor("gam", [128, KC], F32, kind="ExternalInput")
    bet_in = nc.dram_tensor("bet", [128, KC], F32, kind="ExternalInput")
    y_s = nc.dram_tensor("y_s", [TO, D], F32, kind="ExternalOutput")

    xT_v = xT.rearrange("(kc p) t -> p kc t", p=128)
    w1q_v = w1q.rearrange("(kc p) f -> p kc f", p=128)
    w1k_v = w1k.rearrange("(kc p) f -> p kc f", p=128)
    w1v_v = w1v.rearrange("(kc p) f -> p kc f", p=128)
    w1u_v = w1u.rearrange("(kc p) f -> p kc f", p=128)
    w2_v = w2.rearrange("(kc p) n -> p kc n", p=128)

    with tile.TileContext(nc) as tc:
        with (
            tc.tile_pool(name="persist", bufs=1) as sbp,
            tc.tile_pool(name="small", bufs=2) as sbs,
            tc.tile_pool(name="dram", bufs=1, space="DRAM") as dram,
        ):
            # ---- constants / small params
            ident = sbp.tile([128, 128], BF16)
            make_identity(nc, ident[:])
            ones_f = sbp.tile([128, 128], F32)
            nc.vector.memset(ones_f[:], 1.0)
            ones_col = sbp.tile([128, 1], BF16)
            nc.vector.tensor_copy(ones_col[:], ones_f[:, 0:1])
            ones_row = sbp.tile([1, 128], F32R)
            nc.vector.tensor_copy(ones_row[:], ones_f[0:1, :])

            b1q_s = sbp.tile([128, 1], F32)
            b1k_s = sbp.tile([128, 1], F32)
            b1v_s = sbp.tile([128, 1], F32)
            b1u_s = sbp.tile([128, 1], F32)
            b2_s = sbp.tile([128, D], F32)
            gam = sbp.tile([128, KC], F32)
            bet = sbp.tile([128, KC], F32)
            nc.sync.dma_start(b1q_s[:], b1q[:, :])
            nc.sync.dma_start(b1k_s[:], b1k[:, :])
            nc.sync.dma_start(b1v_s[:], b1v[:, :])
            nc.sync.dma_start(b1u_s[:], b1u[:, :])
            nc.sync.dma_start(b2_s[:], b2_bc[:, :])
            nc.sync.dma_start(gam[:], gam_in[:, :])
            nc.sync.dma_start(bet[:], bet_in[:, :])

            # ---- persistent activations (bf16)
            kT = sbp.tile([128, TT], BF16)
            qT = sbp.tile([128, TT], BF16)
            uT = sbp.tile([128, TT], BF16)
            vTok = sbp.tile([128, NTC, 128], BF16)   # v token-major
            gatedT = sbp.tile([128, TT], BF16)
            gfull = sbp.tile([128, KC, TO], BF16)
            normedT = sbp.tile([128, KC, TO], BF16)

            # block-diagonal head-pair packs: every score/AV matmul gets a
            # full 128x128 stationary tile (64-wide tiles run at half rate).
            # kblk2 block c: cols 0:64 = head-A k for kv [128c,+64) on rows
            # 0:64; cols 64:128 = head-B k for kv [128c+64,+128) on rows
            # 64:128. kblk3 is the mirror (B first half / A second half).
            kblk2 = sbp.tile([128, NTC, 128], BF16)
            kblk3 = sbp.tile([128, NTC, 128], BF16)
            vblk1 = sbp.tile([128, NTC, 128], BF16)
            vblk2 = sbp.tile([128, NTC, 128], BF16)

            w1q_t = sbp.tile([128, KC, 128], BF16)
            w1k_t = sbp.tile([128, KC, 128], BF16)
            w1v_t = sbp.tile([128, KC, 128], BF16)
            w1u_t = sbp.tile([128, KC, 128], BF16)
            w2_t = sbp.tile([128, KC, D], BF16)

            a2a_in = dram.tile([8, 128, TO], BF16, name="a2a_in")
            a2a_out = dram.tile([8, 128, TO], BF16, name="a2a_out")

            # ================= stage 1: f1 (k, q, v, u) =================
            with (
                tc.tile_pool(name="xpool", bufs=1) as xp,
                tc.tile_pool(name="ps_f1", bufs=2, space="PSUM") as ps_f1,
                tc.tile_pool(name="ps_tr", bufs=4, space="PSUM") as ps_tr,
            ):
                xTs = xp.tile([128, KC, TT], BF16)   # 64 KB/partition
                # weights first (small), then x quarters, so f1 starts early
                nc.sync.dma_start(w1k_t[:], w1k_v[:, :, :])
                nc.sync.dma_start(w1q_t[:], w1q_v[:, :, :])
                nc.sync.dma_start(w1v_t[:], w1v_v[:, :, :])
                nc.sync.dma_start(w1u_t[:], w1u_v[:, :, :])
                for qh in range(4):
                    nc.sync.dma_start(xTs[:, :, qh * 1024:(qh + 1) * 1024],
                                      xT_v[:, :, qh * 1024:(qh + 1) * 1024])

                nc.vector.memset(kblk2[:], 0.0)
                nc.vector.memset(kblk3[:], 0.0)
                nc.vector.memset(vblk1[:], 0.0)
                nc.vector.memset(vblk2[:], 0.0)
                # feature-major projections: out [128 f, 1024 t] per quarter
                for wt, bt, dst in ((w1k_t, b1k_s, kT), (w1q_t, b1q_s, qT),
                                    (w1v_t, b1v_s, None), (w1u_t, b1u_s, uT)):
                    for qh in range(4):
                        ps = ps_f1.tile([128, 1024], F32, tag="f1")
                        for kc in range(KC):
                            for n2 in range(2):
                                nc.tensor.matmul(
                                    ps[:, n2 * 512:(n2 + 1) * 512], wt[:, kc, :],
                                    xTs[:, kc, qh * 1024 + n2 * 512:qh * 1024 + (n2 + 1) * 512],
                                    start=(kc == 0), stop=(kc == KC - 1))
                        if dst is not None:
                            nc.scalar.activation(
                                dst[:, qh * 1024:(qh + 1) * 1024], ps[:],
                                SILU, bias=bt[:, 0:1], scale=1.0)
                        else:
                            # v: silu into a staging tile, then PE-transpose
                            vf = sbs.tile([128, 1024], BF16, tag="vf")
                            nc.scalar.activation(vf[:], ps[:], SILU,
                                                 bias=bt[:, 0:1], scale=1.0)
                            for t8 in range(8):
                                pt = ps_tr.tile([128, 128], BF16, tag="tr")
                                nc.tensor.transpose(
                                    pt[:], vf[:, t8 * 128:(t8 + 1) * 128],
                                    ident[:])
                                nc.vector.tensor_copy(
                                    vTok[:, qh * 8 + t8, :], pt[:])

                # build the block-diagonal packs (strided bulk copies)
                kTv = kT[:].rearrange("p (c t) -> p c t", t=128)
                nc.vector.tensor_copy(kblk2[0:64, :, 0:64], kTv[0:64, :, 0:64])
                nc.vector.tensor_copy(kblk2[64:128, :, 64:128],
                                      kTv[64:128, :, 64:128])
                nc.vector.tensor_copy(kblk3[0:64, :, 64:128],
                                      kTv[0:64, :, 64:128])
                nc.vector.tensor_copy(kblk3[64:128, :, 0:64],
                                      kTv[64:128, :, 0:64])
                nc.vector.tensor_copy(vblk1[0:64, :, 0:64], vTok[0:64, :, 0:64])
                nc.vector.tensor_copy(vblk1[64:128, :, 64:128],
                                      vTok[64:128, :, 64:128])
                nc.vector.tensor_copy(vblk2[0:64, :, 64:128],
                                      vTok[0:64, :, 64:128])
                nc.vector.tensor_copy(vblk2[64:128, :, 0:64],
                                      vTok[64:128, :, 0:64])

            # ================= stage 2: attention per batch =================
            with (
                tc.tile_pool(name="ps_av", bufs=1, space="PSUM") as ps_av,
                tc.tile_pool(name="ps_s", bufs=2, space="PSUM") as ps_s,
                tc.tile_pool(name="attn", bufs=3) as attn,
            ):
                for bt in range(2):
                    t_b = bt * S
                    av = ps_av.tile([128, 2, 1024], F32, tag="av")
                    for qh in range(2):
                        q0 = t_b + qh * 1024
                        for kt in range(16):
                            c = bt * 16 + kt
                            for blk, vbt in ((kblk2, vblk1), (kblk3, vblk2)):
                                s = ps_s.tile([128, 1024], F32, tag="s")
                                for n2 in range(2):
                                    nc.tensor.matmul(
                                        s[:, n2 * 512:(n2 + 1) * 512],
                                        blk[:, c, :],
                                        qT[:, q0 + n2 * 512:q0 + (n2 + 1) * 512],
                                        start=True, stop=True)
                                a = attn.tile([128, 1024], BF16, tag="a")
                                nc.scalar.activation(a[:], s[:], SILU)
                                for n2 in range(2):
                                    nc.tensor.matmul(
                                        av[:, qh, n2 * 512:(n2 + 1) * 512],
                                        vbt[:, c, :],
                                        a[:, n2 * 512:(n2 + 1) * 512],
                                        start=(kt == 0 and blk is kblk2),
                                        stop=(kt == 15 and blk is kblk3))
                    # gate with u; ship the finished token range to DRAM
                    for qh in range(2):
                        g0 = t_b + qh * 1024
                        nc.vector.tensor_tensor(
                            gatedT[:, g0:g0 + 1024], av[:, qh, :],
                            uT[:, g0:g0 + 1024], MULT)
                        j0 = (bt * 2 + qh) * 2
                        nc.gpsimd.dma_start(a2a_in[j0],
                                            gatedT[:, g0:g0 + TO])
                        nc.gpsimd.dma_start(a2a_in[j0 + 1],
                                            gatedT[:, g0 + TO:g0 + 1024])

            # ================= stage 3: AllToAll + LayerNorm =================
            nc.gpsimd.collective_compute(
                "AllToAll", mybir.AluOpType.bypass,
                replica_groups=[[0, 1, 2, 3, 4, 5, 6, 7]],
                ins=[a2a_in[:]], outs=[a2a_out[:]])
            for j in range(KC):
                nc.sync.dma_start(gfull[:, j, :], a2a_out[j])

            with (
                tc.tile_pool(name="ln", bufs=1) as ln,
                tc.tile_pool(name="ps_ln", bufs=1, space="PSUM") as ps_ln,
            ):
                st_sum = ps_ln.tile([1, TO], F32, tag="st_sum")
                st_sq = ps_ln.tile([1, TO], F32, tag="st_sq")
                for kc in range(KC):
                    nc.tensor.matmul(st_sum[:], ones_col[:], gfull[:, kc, :],
                                     start=(kc == 0), stop=(kc == KC - 1))
                for kc in range(KC):
                    sq = sbs.tile([128, TO], BF16, tag="sq")
                    nc.vector.tensor_tensor(sq[:], gfull[:, kc, :],
                                            gfull[:, kc, :], MULT)
                    nc.tensor.matmul(st_sq[:], ones_col[:], sq[:],
                                     start=(kc == 0), stop=(kc == KC - 1))

                mu = ln.tile([1, TO], F32, tag="mu")
                nc.vector.tensor_scalar_mul(mu[:], st_sum[:], 1.0 / D)
                m2 = ln.tile([1, TO], F32, tag="m2")
                nc.vector.tensor_scalar_mul(m2[:], st_sq[:], 1.0 / D)
                mu2 = ln.tile([1, TO], F32, tag="mu2")
                nc.vector.tensor_tensor(mu2[:], mu[:], mu[:], MULT)
                varE = ln.tile([1, TO], F32, tag="varE")
                nc.vector.tensor_tensor(varE[:], m2[:], mu2[:], SUB)
                nc.vector.tensor_scalar_add(varE[:], varE[:], EPS_EFF)
                std = ln.tile([1, TO], F32, tag="std")
                nc.scalar.activation(std[:], varE[:], SQRT)
                r0 = ln.tile([1, TO], F32, tag="r0")
                nc.vector.reciprocal(r0[:], std[:])
                # one Newton step: r1 = r0 * (1.5 - 0.5 * varE * r0^2)
                nt1 = ln.tile([1, TO], F32, tag="nt1")
                nc.vector.tensor_tensor(nt1[:], r0[:], r0[:], MULT)
                nc.vector.tensor_tensor(nt1[:], nt1[:], varE[:], MULT)
                nc.vector.tensor_scalar(nt1[:], nt1[:], -0.5, 1.5, MULT, ADD)
                rstd = ln.tile([1, TO], F32R, tag="rstd")
                nc.vector.tensor_tensor(rstd[:], r0[:], nt1[:], MULT)
                mu_r = ln.tile([1, TO], F32R, tag="mu_r")
                nc.vector.tensor_copy(mu_r[:], mu[:])

                ps_mu = ps_ln.tile([128, TO], F32, tag="ps_mu")
                ps_r = ps_ln.tile([128, TO], F32, tag="ps_r")
                nc.tensor.matmul(ps_mu[:], ones_row[:], mu_r[:], start=True, stop=True)
                nc.tensor.matmul(ps_r[:], ones_row[:], rstd[:], start=True, stop=True)

                for kc in range(KC):
                    t1 = sbs.tile([128, TO], F32, tag="t1")
                    nc.vector.tensor_tensor(t1[:], gfull[:, kc, :], ps_mu[:], SUB)
                    nc.vector.tensor_tensor(t1[:], t1[:], ps_r[:], MULT)
                    nc.vector.tensor_scalar(normedT[:, kc, :], t1[:],
                                            gam[:, kc:kc + 1], bet[:, kc:kc + 1],
                                            MULT, ADD)

                # load W2 while attention/LN runs
                nc.sync.dma_start(w2_t[:], w2_v[:, :, :])

            # ================= stage 4: f2 + bias + store =================
            with (
                tc.tile_pool(name="ps_y", bufs=2, space="PSUM") as ps_y,
                tc.tile_pool(name="yout", bufs=2) as yout,
            ):
                for tt in range(4):
                    psy = ps_y.tile([128, D], F32, tag="y")
                    for kc in range(KC):
                        for n2 in range(2):
                            nc.tensor.matmul(
                                psy[:, n2 * 512:(n2 + 1) * 512],
                                normedT[:, kc, tt * 128:(tt + 1) * 128],
                                w2_t[:, kc, n2 * 512:(n2 + 1) * 512],
                                start=(kc == 0), stop=(kc == KC - 1))
                    yo = yout.tile([128, D], F32, tag="yo")
                    nc.vector.tensor_tensor(yo[:], psy[:], b2_s[:], ADD)
                    nc.sync.dma_start(y_s[tt * 128:(tt + 1) * 128, :], yo[:])

    nc.compile()
    return nc


def _get_nc():
    if "nc" not in _CACHE:
        _CACHE["nc"] = _build()
    return _CACHE["nc"]


def kernel(x, W1, b1, W2, b2, gamma, beta, **kw):
    nc = _get_nc()
    bf16 = ml_dtypes.bfloat16
    x = np.asarray(x, dtype=np.float32)
    W1 = np.asarray(W1, dtype=np.float32)
    b1 = np.asarray(b1, dtype=np.float32)
    W2bf = np.ascontiguousarray(np.asarray(W2, dtype=np.float32).astype(bf16))
    b2_bc = np.ascontiguousarray(
        np.broadcast_to(np.asarray(b2, dtype=np.float32), (128, D)))
    gam_t = np.ascontiguousarray(
        np.asarray(gamma, dtype=np.float32).reshape(KC, 128).T)
    bet_t = np.ascontiguousarray(
        np.asarray(beta, dtype=np.float32).reshape(KC, 128).T)
    # [D, 4096]: both batches concatenated along tokens
    xTall = np.ascontiguousarray(
        x.transpose(2, 0, 1).reshape(D, TT).astype(bf16))

    in_maps = []
    u0, v0, q0, k0 = 0, D, 2 * D, 3 * D
    for c in range(8):
        cs = 128 * c
        in_maps.append({
            "xT": xTall,
            "w1q": np.ascontiguousarray(W1[:, q0 + cs:q0 + cs + 128].astype(bf16)),
            "w1k": np.ascontiguousarray(W1[:, k0 + cs:k0 + cs + 128].astype(bf16)),
            "w1v": np.ascontiguousarray(W1[:, v0 + cs:v0 + cs + 128].astype(bf16)),
            "w1u": np.ascontiguousarray(W1[:, u0 + cs:u0 + cs + 128].astype(bf16)),
            "b1q": np.ascontiguousarray(b1[q0 + cs:q0 + cs + 128].reshape(128, 1)),
            "b1k": np.ascontiguousarray(b1[k0 + cs:k0 + cs + 128].reshape(128, 1)),
            "b1v": np.ascontiguousarray(b1[v0 + cs:v0 + cs + 128].reshape(128, 1)),
            "b1u": np.ascontiguousarray(b1[u0 + cs:u0 + cs + 128].reshape(128, 1)),
            "w2": W2bf,
            "b2_bc": b2_bc,
            "gam": gam_t,
            "bet": bet_t,
        })
    res = run_bass_kernel_spmd(nc, in_maps, core_ids=list(range(8)), **kw)
    y = np.empty((B, S, D), dtype=np.float32)
    for c in range(8):
        b = c // 4
        t0 = 512 * (c % 4)
        y[b, t0:t0 + 512, :] = res.results[c]["y_s"]
    if kw:
        _CACHE["last_res"] = res
    return y


# revision 29
# speedup vs baseline: 1.7641x; 1.1189x over previous
"""HSTU block kernel for 8 Trainium2 NeuronCores — head-sharded attention.

Sharding: core c owns heads {2c, 2c+1} (a 128-feature slice of each of the
q/k/v/u projections) for ALL 4096 tokens (both batches). f1 computes only
those W1 columns, so attention is fully local per core — no kv exchange.
After gating (av*u, feature-major) one 8-core AllToAll (1 MB) re-shards
features -> tokens; LN + f2 then run token-parallel on 512 tokens per core.

All matmuls run in bf16 (full PE rate, half the DMA/SBUF of fp32). The host
pre-transposes x and pre-slices W1/b1 so the device does no input transposes
and only ~20 fat DMAs. Scores use K=64 contraction with the two heads of a
pair packed on partitions via tile_position; AV accumulates both heads into
one PSUM tile via output tile_position. The silu(scores)/S scaling is folded
into LayerNorm via eps' = S^2 * eps (LN is scale-invariant except for eps).
"""

import sys

sys.path.insert(0, "/opt/trn_rl_repo")

import ml_dtypes
import numpy as np

import concourse.bass as bass
import concourse.mybir as mybir
import concourse.tile as tile
from concourse import bacc
from concourse.bass_utils import run_bass_kernel_spmd
from concourse.masks import make_identity

F32 = mybir.dt.float32
F32R = mybir.dt.float32r
BF16 = mybir.dt.bfloat16
SILU = mybir.ActivationFunctionType.Silu
SQRT = mybir.ActivationFunctionType.Sqrt
MULT = mybir.AluOpType.mult
ADD = mybir.AluOpType.add
SUB = mybir.AluOpType.subtract

B, S, D = 2, 2048, 1024
TT = B * S         # 4096 tokens total (f1/attention scope per core)
TO = 512           # output tokens per core
KC = D // 128      # 8 feature chunks of the model dim
NTC = TT // 128    # 32 token chunks
EPS_EFF = float(S) * float(S) * 1e-5

_CACHE = {}


def _build():
    nc = bacc.Bacc(None, target_bir_lowering=False, num_devices=8)

    xT = nc.dram_tensor("xT", [D, TT], BF16, kind="ExternalInput")
    w1q = nc.dram_tensor("w1q", [D, 128], BF16, kind="ExternalInput")
    w1k = nc.dram_tensor("w1k", [D, 128], BF16, kind="ExternalInput")
    w1v = nc.dram_tensor("w1v", [D, 128], BF16, kind="ExternalInput")
    w1u = nc.dram_tensor("w1u", [D, 128], BF16, kind="ExternalInput")
    b1q = nc.dram_tensor("b1q", [128, 1], F32, kind="ExternalInput")
    b1k = nc.dram_tensor("b1k", [128, 1], F32, kind="ExternalInput")
    b1v = nc.dram_tensor("b1v", [128, 1], F32, kind="ExternalInput")
    b1u = nc.dram_tensor("b1u", [128, 1], F32, kind="ExternalInput")
    w2 = nc.dram_tensor("w2", [D, D], BF16, kind="ExternalInput")   # gamma-scaled
    b2p_in = nc.dram_tensor("b2p_bc", [128, D], F32, kind="ExternalInput")
    c2_in = nc.dram_tensor("c2_bc", [128, D], F32, kind="ExternalInput")
    y_s = nc.dram_tensor("y_s", [TO, D], F32, kind="ExternalOutput")

    xT_v = xT.rearrange("(kc p) t -> p kc t", p=128)
    w1q_v = w1q.rearrange("(kc p) f -> p kc f", p=128)
    w1k_v = w1k.rearrange("(kc p) f -> p kc f", p=128)
    w1v_v = w1v.rearrange("(kc p) f -> p kc f", p=128)
    w1u_v = w1u.rearrange("(kc p) f -> p kc f", p=128)
    w2_v = w2.rearrange("(kc p) n -> p kc n", p=128)

    with tile.TileContext(nc) as tc:
        with (
            tc.tile_pool(name="persist", bufs=1) as sbp,
            tc.tile_pool(name="small", bufs=2) as sbs,
            tc.tile_pool(name="dram", bufs=1, space="DRAM") as dram,
        ):
            # ---- constants / small params
            ident = sbp.tile([128, 128], BF16)
            make_identity(nc, ident[:])
            # stats lhsT pre-scaled by 1/D so the matmul yields means directly
            ones_col = sbp.tile([128, 1], BF16)
            nc.vector.memset(ones_col[:], 1.0 / D)
            one_f = sbp.tile([1, 1], F32)
            nc.vector.memset(one_f[:], 1.0)

            b1q_s = sbp.tile([128, 1], F32)
            b1k_s = sbp.tile([128, 1], F32)
            b1v_s = sbp.tile([128, 1], F32)
            b1u_s = sbp.tile([128, 1], F32)
            b2p_s = sbp.tile([128, D], F32)
            c2_s = sbp.tile([128, D], F32)
            eps_t = sbp.tile([128, 1], F32)
            nc.vector.memset(eps_t[:], EPS_EFF)
            nc.sync.dma_start(b1q_s[:], b1q[:, :])
            nc.sync.dma_start(b1k_s[:], b1k[:, :])
            nc.sync.dma_start(b1v_s[:], b1v[:, :])
            nc.sync.dma_start(b1u_s[:], b1u[:, :])

            # ---- persistent activations (bf16)
            kT = sbp.tile([128, TT], BF16)
            qT = sbp.tile([128, TT], BF16)
            uT = sbp.tile([128, TT], BF16)
            vTok = sbp.tile([128, NTC, 128], BF16)   # v token-major
            gatedT = sbp.tile([128, TT], BF16)
            gfull = sbp.tile([128, KC, TO], BF16)

            # block-diagonal head-pair packs: every score/AV matmul gets a
            # full 128x128 stationary tile (64-wide tiles run at half rate).
            # kblk2 block c: cols 0:64 = head-A k for kv [128c,+64) on rows
            # 0:64; cols 64:128 = head-B k for kv [128c+64,+128) on rows
            # 64:128. kblk3 is the mirror (B first half / A second half).
            kblk2 = sbp.tile([128, NTC, 128], BF16)
            kblk3 = sbp.tile([128, NTC, 128], BF16)
            vblk1 = sbp.tile([128, NTC, 128], BF16)
            vblk2 = sbp.tile([128, NTC, 128], BF16)

            w1q_t = sbp.tile([128, KC, 128], BF16)
            w1k_t = sbp.tile([128, KC, 128], BF16)
            w1v_t = sbp.tile([128, KC, 128], BF16)
            w1u_t = sbp.tile([128, KC, 128], BF16)
            w2_t = sbp.tile([128, KC, D], BF16)

            a2a_in = dram.tile([8, 128, TO], BF16, name="a2a_in")
            a2a_out = dram.tile([8, 128, TO], BF16, name="a2a_out")

            # ================= stage 1: f1 (k, q, v, u) =================
            with (
                tc.tile_pool(name="xpool", bufs=1) as xp,
                tc.tile_pool(name="ps_f1", bufs=2, space="PSUM") as ps_f1,
                tc.tile_pool(name="ps_tr", bufs=4, space="PSUM") as ps_tr,
            ):
                xTs = xp.tile([128, KC, TT], BF16)   # 64 KB/partition
                # weights first (small), then x quarters, so f1 starts early
                nc.sync.dma_start(w1k_t[:], w1k_v[:, :, :])
                nc.sync.dma_start(w1q_t[:], w1q_v[:, :, :])
                nc.sync.dma_start(w1v_t[:], w1v_v[:, :, :])
                nc.sync.dma_start(w1u_t[:], w1u_v[:, :, :])
                for qh in range(4):
                    nc.sync.dma_start(xTs[:, :, qh * 1024:(qh + 1) * 1024],
                                      xT_v[:, :, qh * 1024:(qh + 1) * 1024])

                nc.vector.memset(kblk2[:], 0.0)
                nc.vector.memset(kblk3[:], 0.0)
                nc.vector.memset(vblk1[:], 0.0)
                nc.vector.memset(vblk2[:], 0.0)
                # feature-major projections: out [128 f, 1024 t] per quarter
                for wt, bt, dst in ((w1k_t, b1k_s, kT), (w1q_t, b1q_s, qT),
                                    (w1v_t, b1v_s, None), (w1u_t, b1u_s, uT)):
                    for qh in range(4):
                        ps = ps_f1.tile([128, 1024], F32, tag="f1")
                        for kc in range(KC):
                            for n2 in range(2):
                                nc.tensor.matmul(
                                    ps[:, n2 * 512:(n2 + 1) * 512], wt[:, kc, :],
                                    xTs[:, kc, qh * 1024 + n2 * 512:qh * 1024 + (n2 + 1) * 512],
                                    start=(kc == 0), stop=(kc == KC - 1))
                        if dst is not None:
                            nc.scalar.activation(
                                dst[:, qh * 1024:(qh + 1) * 1024], ps[:],
                                SILU, bias=bt[:, 0:1], scale=1.0)
                        else:
                            # v: silu into a staging tile, then PE-transpose
                            vf = sbs.tile([128, 1024], BF16, tag="vf")
                            nc.scalar.activation(vf[:], ps[:], SILU,
                                                 bias=bt[:, 0:1], scale=1.0)
                            for t8 in range(8):
                                pt = ps_tr.tile([128, 128], BF16, tag="tr")
                                nc.tensor.transpose(
                                    pt[:], vf[:, t8 * 128:(t8 + 1) * 128],
                                    ident[:])
                                nc.vector.tensor_copy(
                                    vTok[:, qh * 8 + t8, :], pt[:])

                # build the block-diagonal packs (strided bulk copies)
                kTv = kT[:].rearrange("p (c t) -> p c t", t=128)
                nc.vector.tensor_copy(kblk2[0:64, :, 0:64], kTv[0:64, :, 0:64])
                nc.vector.tensor_copy(kblk2[64:128, :, 64:128],
                                      kTv[64:128, :, 64:128])
                nc.vector.tensor_copy(kblk3[0:64, :, 64:128],
                                      kTv[0:64, :, 64:128])
                nc.vector.tensor_copy(kblk3[64:128, :, 0:64],
                                      kTv[64:128, :, 0:64])
                nc.vector.tensor_copy(vblk1[0:64, :, 0:64], vTok[0:64, :, 0:64])
                nc.vector.tensor_copy(vblk1[64:128, :, 64:128],
                                      vTok[64:128, :, 64:128])
                nc.vector.tensor_copy(vblk2[0:64, :, 64:128],
                                      vTok[0:64, :, 64:128])
                nc.vector.tensor_copy(vblk2[64:128, :, 0:64],
                                      vTok[64:128, :, 0:64])

            # f2 params arrive during attention (DMA queues are idle then)
            nc.sync.dma_start(w2_t[:], w2_v[:, :, :])
            nc.sync.dma_start(b2p_s[:], b2p_in[:, :])
            nc.sync.dma_start(c2_s[:], c2_in[:, :])

            # ================= stage 2: attention per batch =================
            with (
                tc.tile_pool(name="ps_av", bufs=1, space="PSUM") as ps_av,
                tc.tile_pool(name="ps_s", bufs=2, space="PSUM") as ps_s,
                tc.tile_pool(name="attn", bufs=3) as attn,
            ):
                for bt in range(2):
                    t_b = bt * S
                    av = ps_av.tile([128, 2, 1024], F32, tag="av")
                    for qh in range(2):
                        q0 = t_b + qh * 1024
                        for kt in range(16):
                            c = bt * 16 + kt
                            for blk, vbt in ((kblk2, vblk1), (kblk3, vblk2)):
                                s = ps_s.tile([128, 1024], F32, tag="s")
                                for n2 in range(2):
                                    nc.tensor.matmul(
                                        s[:, n2 * 512:(n2 + 1) * 512],
                                        blk[:, c, :],
                                        qT[:, q0 + n2 * 512:q0 + (n2 + 1) * 512],
                                        start=True, stop=True)
                                a = attn.tile([128, 1024], BF16, tag="a")
                                nc.scalar.activation(a[:], s[:], SILU)
                                for n2 in range(2):
                                    nc.tensor.matmul(
                                        av[:, qh, n2 * 512:(n2 + 1) * 512],
                                        vbt[:, c, :],
                                        a[:, n2 * 512:(n2 + 1) * 512],
                                        start=(kt == 0 and blk is kblk2),
                                        stop=(kt == 15 and blk is kblk3))
                    # gate with u; ship the finished token range to DRAM
                    for qh in range(2):
                        g0 = t_b + qh * 1024
                        nc.vector.tensor_tensor(
                            gatedT[:, g0:g0 + 1024], av[:, qh, :],
                            uT[:, g0:g0 + 1024], MULT)
                        j0 = (bt * 2 + qh) * 2
                        nc.gpsimd.dma_start(a2a_in[j0],
                                            gatedT[:, g0:g0 + TO])
                        nc.gpsimd.dma_start(a2a_in[j0 + 1],
                                            gatedT[:, g0 + TO:g0 + 1024])

            # ================= stage 3: AllToAll + LN stats =================
            # LN is folded into f2: y = r*(G - mu*c2) + b2', where
            # G = g @ (gamma*W2), c2 = gamma@W2 (bc), b2' = beta@W2+b2 (bc).
            nc.gpsimd.collective_compute(
                "AllToAll", mybir.AluOpType.bypass,
                replica_groups=[[0, 1, 2, 3, 4, 5, 6, 7]],
                ins=[a2a_in[:]], outs=[a2a_out[:]])
            for j in range(KC):
                nc.sync.dma_start(gfull[:, j, :], a2a_out[j])

            rcol = sbp.tile([128, 4], F32)
            rmcol = sbp.tile([128, 4], F32)
            with (
                tc.tile_pool(name="ln", bufs=1) as ln,
                tc.tile_pool(name="ps_ln", bufs=1, space="PSUM") as ps_ln,
            ):
                st_sum = ps_ln.tile([1, TO], F32, tag="st_sum")
                st_sq = ps_ln.tile([1, TO], F32, tag="st_sq")
                for kc in range(KC):
                    nc.tensor.matmul(st_sum[:], ones_col[:], gfull[:, kc, :],
                                     start=(kc == 0), stop=(kc == KC - 1))
                for kc in range(KC):
                    sq = sbs.tile([128, TO], BF16, tag="sq")
                    nc.vector.tensor_tensor(sq[:], gfull[:, kc, :],
                                            gfull[:, kc, :], MULT)
                    nc.tensor.matmul(st_sq[:], ones_col[:], sq[:],
                                     start=(kc == 0), stop=(kc == KC - 1))

                # move stats into a [128, 4] column layout (t = tt*128 + p)
                # so the whole scalar chain (incl. the 8-cyc/elem reciprocal)
                # runs across 128 lanes instead of one.
                mu_row = ln.tile([1, TO], F32, tag="mu_row")
                m2_row = ln.tile([1, TO], F32, tag="m2_row")
                nc.vector.tensor_copy(mu_row[:], st_sum[:])
                nc.vector.tensor_copy(m2_row[:], st_sq[:])
                # PE-transpose the stat rows into column layout [128, 4]
                stt = ps_ln.tile([128, 8], F32, tag="stt")
                for tt in range(4):
                    nc.tensor.transpose(stt[:, tt:tt + 1],
                                        mu_row[0:1, tt * 128:(tt + 1) * 128],
                                        one_f[0:1, 0:1])
                    nc.tensor.transpose(stt[:, 4 + tt:5 + tt],
                                        m2_row[0:1, tt * 128:(tt + 1) * 128],
                                        one_f[0:1, 0:1])
                mu_c = ln.tile([128, 4], F32, tag="mu_c")
                m2_c = ln.tile([128, 4], F32, tag="m2_c")
                nc.vector.tensor_copy(mu_c[:], stt[:, 0:4])
                nc.vector.tensor_copy(m2_c[:], stt[:, 4:8])

                varE = ln.tile([128, 4], F32, tag="varE")
                nc.vector.tensor_tensor(varE[:], mu_c[:], mu_c[:], MULT)
                nc.vector.tensor_tensor(varE[:], m2_c[:], varE[:], SUB)
                std = ln.tile([128, 4], F32, tag="std")
                nc.scalar.activation(std[:], varE[:], SQRT, bias=eps_t[:, 0:1],
                                     scale=1.0)
                r0 = ln.tile([128, 4], F32, tag="r0")
                nc.vector.reciprocal(r0[:], std[:])
                # one Newton step: r = r0 * (1.5 - 0.5 * (varE+eps) * r0^2)
                vpe = ln.tile([128, 4], F32, tag="vpe")
                nc.vector.tensor_scalar(vpe[:], varE[:], eps_t[:, 0:1], -0.5,
                                        ADD, MULT)
                nt1 = ln.tile([128, 4], F32, tag="nt1")
                nc.vector.tensor_tensor(nt1[:], r0[:], r0[:], MULT)
                nc.vector.tensor_tensor(nt1[:], nt1[:], vpe[:], MULT)
                nc.vector.tensor_scalar_add(nt1[:], nt1[:], 1.5)
                nc.vector.tensor_tensor(rcol[:], r0[:], nt1[:], MULT)
                nc.vector.tensor_tensor(rmcol[:], rcol[:], mu_c[:], MULT)

            # ================= stage 4: f2 + LN post-ops + store =================
            with (
                tc.tile_pool(name="ps_y", bufs=1, space="PSUM") as ps_y,
                tc.tile_pool(name="yout", bufs=2) as yout,
            ):
                for tt in range(4):
                    psy = ps_y.tile([128, D], F32, tag=f"y{tt}", name=f"psy{tt}")
                    for kc in range(KC):
                        for n2 in range(2):
                            nc.tensor.matmul(
                                psy[:, n2 * 512:(n2 + 1) * 512],
                                gfull[:, kc, tt * 128:(tt + 1) * 128],
                                w2_t[:, kc, n2 * 512:(n2 + 1) * 512],
                                start=(kc == 0), stop=(kc == KC - 1))
                    # y = psy * r - (c2 * r*mu - b2')
                    mterm = yout.tile([128, D], F32, tag="mt")
                    nc.vector.tensor_scalar(mterm[:], c2_s[:],
                                            rmcol[:, tt:tt + 1], 0.0, MULT, ADD)
                    nc.vector.tensor_tensor(mterm[:], mterm[:], b2p_s[:], SUB)
                    yo = yout.tile([128, D], F32, tag="yo")
                    nc.vector.tensor_scalar(yo[:], psy[:],
                                            rcol[:, tt:tt + 1], 0.0, MULT, ADD)
                    nc.vector.tensor_tensor(yo[:], yo[:], mterm[:], SUB)
                    nc.sync.dma_start(y_s[tt * 128:(tt + 1) * 128, :], yo[:])

    nc.compile()
    return nc


def _get_nc():
    if "nc" not in _CACHE:
        _CACHE["nc"] = _build()
    return _CACHE["nc"]


def kernel(x, W1, b1, W2, b2, gamma, beta, **kw):
    nc = _get_nc()
    bf16 = ml_dtypes.bfloat16
    x = np.asarray(x, dtype=np.float32)
    W1 = np.asarray(W1, dtype=np.float32)
    b1 = np.asarray(b1, dtype=np.float32)
    W2f = np.asarray(W2, dtype=np.float32)
    gamma = np.asarray(gamma, dtype=np.float32)
    beta = np.asarray(beta, dtype=np.float32)
    b2 = np.asarray(b2, dtype=np.float32)
    # LN folded into f2: G = g @ (gamma*W2); y = r*G - (r*mu)*c2 + b2p
    W2bf = np.ascontiguousarray((W2f * gamma[:, None]).astype(bf16))
    c2 = gamma @ W2f
    b2p = beta @ W2f + b2
    c2_bc = np.ascontiguousarray(np.broadcast_to(c2, (128, D)), dtype=np.float32)
    b2p_bc = np.ascontiguousarray(np.broadcast_to(b2p, (128, D)), dtype=np.float32)
    # [D, 4096]: both batches concatenated along tokens
    xTall = np.ascontiguousarray(
        x.transpose(2, 0, 1).reshape(D, TT).astype(bf16))

    in_maps = []
    u0, v0, q0, k0 = 0, D, 2 * D, 3 * D
    for c in range(8):
        cs = 128 * c
        in_maps.append({
            "xT": xTall,
            "w1q": np.ascontiguousarray(W1[:, q0 + cs:q0 + cs + 128].astype(bf16)),
            "w1k": np.ascontiguousarray(W1[:, k0 + cs:k0 + cs + 128].astype(bf16)),
            "w1v": np.ascontiguousarray(W1[:, v0 + cs:v0 + cs + 128].astype(bf16)),
            "w1u": np.ascontiguousarray(W1[:, u0 + cs:u0 + cs + 128].astype(bf16)),
            "b1q": np.ascontiguousarray(b1[q0 + cs:q0 + cs + 128].reshape(128, 1)),
            "b1k": np.ascontiguousarray(b1[k0 + cs:k0 + cs + 128].reshape(128, 1)),
            "b1v": np.ascontiguousarray(b1[v0 + cs:v0 + cs + 128].reshape(128, 1)),
            "b1u": np.ascontiguousarray(b1[u0 + cs:u0 + cs + 128].reshape(128, 1)),
            "w2": W2bf,
            "b2p_bc": b2p_bc,
            "c2_bc": c2_bc,
        })
    res = run_bass_kernel_spmd(nc, in_maps, core_ids=list(range(8)), **kw)
    y = np.empty((B, S, D), dtype=np.float32)
    for c in range(8):
        b = c // 4
        t0 = 512 * (c % 4)
        y[b, t0:t0 + 512, :] = res.results[c]["y_s"]
    if kw:
        _CACHE["last_res"] = res
    return y
